# revision 20
# baseline (speedup 1.0000x reference)
"""Trainium2 Bass kernel for nn_MixtureOfGranularities (MoE with multi-scale routing).

Strategy (8 NeuronCores, SPMD single NEFF):
  - FFN-dim (F=2048) sharded 8 ways: each core holds wg/wu/wd slices [*, :, 256]
    for all 12 experts -> no weight replication (28 MB/core HBM traffic).
  - Router + softmax + top-2 + combine computed on every core (tiny, fp32 for
    bit-stable expert selection).
  - Pooling projections (proj_w4 / proj_w16) sharded by output-dim (96 rows per
    core), AllGather'd to every core.
  - Expert matmuls in float32r (tf32-like, 4x faster than fp32 on the PE).
  - Per-core partial outputs (sum over the local F shard) combined with an
    fp32 ReduceScatter; host concatenates the 8 disjoint shards.
"""
import numpy as np

import concourse.bass as bass
import concourse.tile as tile
from concourse import bacc, mybir
from concourse import bass_utils

F32 = mybir.dt.float32
F32R = mybir.dt.float32r
AFT = mybir.ActivationFunctionType
Alu = mybir.AluOpType
AxX = mybir.AxisListType.X

B, T, D, F, S, EPC, E = 2, 1024, 768, 2048, 3, 4, 12
BT = B * T                  # 2048
NCORE = 8
FS = F // NCORE             # 256  per-core FFN shard
NFH = FS // 128             # 2
DK = D // 128               # 6    d k-tiles
NT = 4                      # token tiles of 512
TT = BT // NT               # 512
DSH = D // NCORE            # 96   pooling dout shard rows per core
G4, G16 = BT // 4, BT // 16  # 512, 128 pooled group counts

_CACHE = {}


def _build():
    nc = bacc.Bacc("TRN2", target_bir_lowering=False, debug=False,
                   enable_asserts=False, num_devices=NCORE)

    xT_d = nc.dram_tensor("xT", [D, BT], F32R, kind="ExternalInput").ap()
    rw_d = nc.dram_tensor("rw", [D, E], F32, kind="ExternalInput").ap()
    pw4_d = nc.dram_tensor("pw4", [4 * D, DSH], F32R, kind="ExternalInput").ap()
    pw16_d = nc.dram_tensor("pw16", [16 * D, DSH], F32R, kind="ExternalInput").ap()
    wg_d = nc.dram_tensor("wg", [E, DK, 128, FS], F32R, kind="ExternalInput").ap()
    wu_d = nc.dram_tensor("wu", [E, DK, 128, FS], F32R, kind="ExternalInput").ap()
    wd_d = nc.dram_tensor("wd", [E, NFH, 128, D], F32R, kind="ExternalInput").ap()
    esel_d = nc.dram_tensor("esel", [E, E * 128], F32R, kind="ExternalInput").ap()
    onesc_d = nc.dram_tensor("onesc", [128, 1], F32, kind="ExternalInput").ap()
    ident_d = nc.dram_tensor("ident", [128, 128], F32, kind="ExternalInput").ap()

    out_d = nc.dram_tensor("out", [DSH, BT], F32, kind="ExternalOutput").ap()
    aux_d = nc.dram_tensor("aux", [1, 1], F32, kind="ExternalOutput").ap()

    with tile.TileContext(nc) as tc:
        _emit(nc, tc, xT_d, rw_d, pw4_d, pw16_d, wg_d, wu_d, wd_d,
              esel_d, onesc_d, ident_d, out_d, aux_d)
    nc.compile()
    return nc


def _emit(nc, tc, xT_d, rw_d, pw4_d, pw16_d, wg_d, wu_d, wd_d,
          esel_d, onesc_d, ident_d, out_d, aux_d):
    persist = tc.alloc_tile_pool(name="persist", bufs=1)
    dram = tc.alloc_tile_pool(name="dram", bufs=1, space="DRAM")

    cT_sb = persist.tile([E, BT], F32R)                  # combine^T [12, 2048]
    acc_sb = persist.tile([128, DK, BT], F32)            # partial out^T  6.29 MB
    esel_sb = persist.tile([E, E * 128], F32R)
    nc.sync.dma_start(esel_sb[:], esel_d)
    r4_sb = persist.tile([128, DK, G4], F32R)
    r16_sb = persist.tile([128, DK, G16], F32R)

    r4part = dram.tile([DSH, G4], F32R)
    r16part = dram.tile([DSH, G16], F32R)
    r4full = dram.tile([D, G4], F32R, addr_space="Shared")
    r16full = dram.tile([D, G16], F32R, addr_space="Shared")

    # ============ PHASE R+P: router / combine / aux / pooling (xT resident) ====
    with tc.tile_pool(name="rp_sb", bufs=3) as rsb, \
         tc.tile_pool(name="rp_ps", bufs=2, space="PSUM") as rps:
        xT_sb = rsb.tile([128, DK, BT], F32R, bufs=1)    # 48 KB/part, phase-local
        for k in range(DK):
            nc.sync.dma_start(xT_sb[:, k], xT_d.rearrange("(k p) t -> p k t", p=128)[:, k])
        xTf32 = rsb.tile([128, DK, BT], F32, bufs=1)     # exact fp32 copy for router
        for k in range(DK):
            nc.sync.dma_start(xTf32[:, k],
                              xT_d.bitcast(F32).rearrange("(k p) t -> p k t", p=128)[:, k])
        rw_sb = rsb.tile([128, DK, E], F32, bufs=1)
        nc.sync.dma_start(rw_sb[:], rw_d.rearrange("(k p) e -> p k e", p=128))
        onesc_sb = rsb.tile([128, 1], F32, bufs=1)
        nc.sync.dma_start(onesc_sb[:], onesc_d)
        ident_sb = rsb.tile([128, 128], F32, bufs=1)
        nc.sync.dma_start(ident_sb[:], ident_d)

        # ---- pooling (dout rows sharded per core; data differs per core) ----
        # batched weight loads: 1 DMA for pw4, 4 chunked DMAs for pw16
        pw4_sb = rsb.tile([128, 4 * DK, DSH], F32R, bufs=1)
        nc.scalar.dma_start(pw4_sb[:], pw4_d.rearrange("(kt p) d -> p kt d", p=128))
        r4_ps = rps.tile([DSH, G4], F32, bufs=1)
        for kt in range(4 * DK):
            j, k = kt // DK, kt % DK
            nc.tensor.matmul(r4_ps[:], pw4_sb[:, kt], xT_sb[:, k, j::4],
                             start=(kt == 0), stop=(kt == 4 * DK - 1))
        r4o = rsb.tile([DSH, G4], F32R, tag="r4o")
        nc.vector.tensor_copy(r4o[:], r4_ps[:])
        nc.sync.dma_start(r4part[:], r4o[:])

        r16_ps = rps.tile([DSH, G16], F32, bufs=1)
        for c in range(8):
            pw16_sb = rsb.tile([128, 2 * DK, DSH], F32R, tag="pw16", bufs=2,
                               name=f"pw16_{c}")
            nc.scalar.dma_start(
                pw16_sb[:], pw16_d.rearrange("(kt p) d -> p kt d", p=128)
                [:, c * 2 * DK:(c + 1) * 2 * DK])
            for kk in range(2 * DK):
                kt = c * 2 * DK + kk
                j, k = kt // DK, kt % DK
                nc.tensor.matmul(r16_ps[:], pw16_sb[:, kk], xT_sb[:, k, j::16],
                                 start=(kt == 0), stop=(kt == 16 * DK - 1))
        r16o = rsb.tile([DSH, G16], F32R, tag="r16o")
        nc.vector.tensor_copy(r16o[:], r16_ps[:])
        nc.sync.dma_start(r16part[:], r16o[:])

        # --- logits for all 2048 tokens into one psum bank [128, 16*12] ---
        lg_ps = rps.tile([128, 16 * E], F32, bufs=1)
        for i in range(16):                              # token tiles of 128
            for k in range(DK):
                nc.tensor.matmul(lg_ps[:, i * E:(i + 1) * E],
                                 xTf32[:, k, i * 128:(i + 1) * 128],
                                 rw_sb[:, k], start=(k == 0), stop=(k == DK - 1))
        lg = rsb.tile([128, 16, E], F32, bufs=1)
        nc.vector.tensor_copy(lg[:], lg_ps[:].rearrange("p (i e) -> p i e", e=E))

        def bc(t2):  # [128,16] -> [128,16,12] free-axis broadcast
            return t2[:].unsqueeze(-1).broadcast_to([128, 16, E])

        # --- batched top-2 on logits ---
        m1 = rsb.tile([128, 16], F32, bufs=1)
        nc.vector.tensor_reduce(m1[:].unsqueeze(-1), lg[:], axis=AxX, op=Alu.max)
        eqm = rsb.tile([128, 16, E], F32, bufs=1)
        nc.vector.tensor_tensor(eqm[:], lg[:], bc(m1), op=Alu.is_equal)
        nc.vector.tensor_scalar_mul(eqm[:], eqm[:], -1e9)
        nc.vector.tensor_add(eqm[:], lg[:], eqm[:])      # masked logits
        m2 = rsb.tile([128, 16], F32, bufs=1)
        nc.vector.tensor_reduce(m2[:].unsqueeze(-1), eqm[:], axis=AxX, op=Alu.max)
        sel = rsb.tile([128, 16, E], F32, bufs=1)
        nc.vector.tensor_tensor(sel[:], lg[:], bc(m2), op=Alu.is_ge)
        # --- batched softmax-free combine: E*sel / sum(E*sel) ---
        ex = rsb.tile([128, 16, E], F32, bufs=1)
        nc.scalar.activation(ex[:], lg[:], AFT.Exp)
        esel = rsb.tile([128, 16, E], F32, bufs=1)
        nc.vector.tensor_mul(esel[:], ex[:], sel[:])
        den = rsb.tile([128, 16], F32, bufs=1)
        nc.vector.tensor_reduce(den[:].unsqueeze(-1), esel[:], axis=AxX, op=Alu.add)
        rden = rsb.tile([128, 16], F32, bufs=1)
        nc.vector.reciprocal(rden[:], den[:])
        cmb = rsb.tile([128, 16, E], F32, bufs=1)
        nc.vector.tensor_tensor(cmb[:], esel[:], bc(rden), op=Alu.mult)
        # gate p for aux load stats
        sume = rsb.tile([128, 16], F32, bufs=1)
        nc.vector.tensor_reduce(sume[:].unsqueeze(-1), ex[:], axis=AxX, op=Alu.add)
        rsum = rsb.tile([128, 16], F32, bufs=1)
        nc.vector.reciprocal(rsum[:], sume[:])
        p_all = rsb.tile([128, 16, E], F32, bufs=1)
        nc.vector.tensor_tensor(p_all[:], ex[:], bc(rsum), op=Alu.mult)

        # --- aux = E/(4096*2048) * sum_e (count_e * gatesum_e) ---
        aux_ps = rps.tile([1, 2, 16 * E], F32, bufs=1)
        nc.tensor.matmul(aux_ps[0:1, 0], onesc_sb[:],
                         sel[:].rearrange("p i e -> p (i e)"), start=True, stop=True)
        nc.tensor.matmul(aux_ps[0:1, 1], onesc_sb[:],
                         p_all[:].rearrange("p i e -> p (i e)"), start=True, stop=True)
        auxv = rsb.tile([1, 2, E], F32, bufs=1)
        nc.vector.tensor_reduce(auxv[:].unsqueeze(-1),
                                aux_ps[:].rearrange("o t (i e) -> o t e i", e=E),
                                axis=AxX, op=Alu.add)
        auxm = rsb.tile([1, E], F32, bufs=1)
        nc.vector.tensor_mul(auxm[:], auxv[:, 0], auxv[:, 1])
        auxs = rsb.tile([1, 1], F32, bufs=1)
        nc.vector.tensor_reduce(auxs[:], auxm[:], axis=AxX, op=Alu.add)
        nc.vector.tensor_scalar_mul(auxs[:], auxs[:], float(E) / (BT * 2 * BT))
        nc.sync.dma_start(aux_d, auxs[:])

        # --- transpose combine -> cT [12, 2048] ---
        for i in range(16):
            ct_ps = rps.tile([E, 128], F32, tag="ct")
            nc.tensor.transpose(ct_ps[:], cmb[:, i], ident_sb[:])
            nc.vector.tensor_copy(cT_sb[:, i * 128:(i + 1) * 128], ct_ps[:])


    nc.gpsimd.collective_compute(
        "AllGather", Alu.bypass, replica_groups=[list(range(NCORE))],
        ins=[r4part.opt()], outs=[r4full.opt()])
    nc.gpsimd.collective_compute(
        "AllGather", Alu.bypass, replica_groups=[list(range(NCORE))],
        ins=[r16part.opt()], outs=[r16full.opt()])
    nc.sync.dma_start(r4_sb[:], r4full[:].rearrange("(k p) g -> p k g", p=128))
    nc.sync.dma_start(r16_sb[:], r16full[:].rearrange("(k p) g -> p k g", p=128))

    # ================= PHASE E: experts =================
    with tc.tile_pool(name="w_sb", bufs=1) as wsb, \
         tc.tile_pool(name="e_sb", bufs=2) as esb, \
         tc.tile_pool(name="e_ps", bufs=2, space="PSUM") as eps:
        for s in range(S):
            wg_t, wu_t, wd_t = [], [], []
            for e in range(EPC):
                eg = s * EPC + e
                wg_sb = wsb.tile([128, DK, FS], F32R, tag="wg", bufs=4, name=f"wg{eg}")
                nc.scalar.dma_start(wg_sb[:], wg_d[eg].rearrange("k p f -> p k f"))
                wu_sb = wsb.tile([128, DK, FS], F32R, tag="wu", bufs=4, name=f"wu{eg}")
                nc.scalar.dma_start(wu_sb[:], wu_d[eg].rearrange("k p f -> p k f"))
                wd_sb = wsb.tile([128, NFH, D], F32R, tag="wd", bufs=4, name=f"wd{eg}")
                nc.sync.dma_start(wd_sb[:], wd_d[eg].rearrange("k p d -> p k d"))
                wg_t.append(wg_sb); wu_t.append(wu_sb); wd_t.append(wd_sb)

            for t in range(NT):
                # moving operand [128, 512] for this scale/k-tile/token-tile
                if s == 0:
                    xtt = esb.tile([128, DK, TT], F32R, tag="xtt", name=f"xtt{t}")
                    nc.sync.dma_start(
                        xtt[:], xT_d
                        .rearrange("(k p) t -> p k t", p=128)[:, :, t * TT:(t + 1) * TT])
                    rhs = lambda k: xtt[:, k]
                elif s == 1:
                    rhs = lambda k: (r4_sb[:, k, t * 128:(t + 1) * 128]
                                     .unsqueeze(-1).broadcast_to([128, 128, 4]))
                else:
                    rhs = lambda k: (r16_sb[:, k, t * 32:(t + 1) * 32]
                                     .unsqueeze(-1).broadcast_to([128, 32, 16]))

                h_t = []
                for e in range(EPC):
                    eg = s * EPC + e
                    cb_ps = eps.tile([128, TT], F32, tag="cb")
                    nc.tensor.matmul(cb_ps[:], esel_sb[:, eg * 128:(eg + 1) * 128],
                                     cT_sb[:, t * TT:(t + 1) * TT], start=True, stop=True)
                    h_sb = wsb.tile([128, NFH, TT], F32R, tag="h", bufs=5, name=f"h{eg}")
                    h_t.append(h_sb)
                    for fh in range(NFH):
                        g_ps = eps.tile([128, TT], F32, tag="gu", bufs=4, name="g_ps")
                        for k in range(DK):
                            nc.tensor.matmul(g_ps[:], wg_t[e][:, k, fh * 128:(fh + 1) * 128],
                                             rhs(k), start=(k == 0), stop=(k == DK - 1))
                        sg = esb.tile([128, TT], F32, tag="sg")
                        nc.scalar.activation(sg[:], g_ps[:], AFT.Silu)
                        u_ps = eps.tile([128, TT], F32, tag="gu", bufs=4, name="u_ps")
                        for k in range(DK):
                            nc.tensor.matmul(u_ps[:], wu_t[e][:, k, fh * 128:(fh + 1) * 128],
                                             rhs(k), start=(k == 0), stop=(k == DK - 1))
                        t1 = esb.tile([128, TT], F32, tag="t1")
                        nc.vector.tensor_mul(t1[:], sg[:], u_ps[:])
                        nc.vector.tensor_mul(h_sb[:, fh], t1[:], cb_ps[:])
                # down projection accumulated over the 4 experts of this scale
                for dt in range(DK):
                    y_ps = eps.tile([128, TT], F32, tag="y")
                    n_mm = EPC * NFH
                    i_mm = 0
                    for e in range(EPC):
                        for k2 in range(NFH):
                            nc.tensor.matmul(y_ps[:], wd_t[e][:, k2, dt * 128:(dt + 1) * 128],
                                             h_t[e][:, k2],
                                             start=(i_mm == 0), stop=(i_mm == n_mm - 1))
                            i_mm += 1
                    dst = acc_sb[:, dt, t * TT:(t + 1) * TT]
                    if s == 0:
                        nc.vector.tensor_copy(dst, y_ps[:])
                    else:
                        nc.vector.tensor_add(dst, dst, y_ps[:])

    # ================= PHASE OUT: chunked ReduceScatter + store =================
    for t in range(NT):
        accT_c = dram.tile([D, TT], F32, tag="accT", name=f"accT{t}", bufs=NT)
        nc.sync.dma_start(accT_c[:].rearrange("(k p) t -> p k t", p=128),
                          acc_sb[:, :, t * TT:(t + 1) * TT])
        rs_c = dram.tile([DSH, TT], F32, tag="rs_c", name=f"rs{t}", bufs=NT)
        nc.gpsimd.collective_compute(
            "ReduceScatter", Alu.add, replica_groups=[list(range(NCORE))],
            ins=[accT_c.opt()], outs=[rs_c.opt()])
        nc.sync.dma_start(out_d[:, t * TT:(t + 1) * TT], rs_c[:])

    persist.release()
    dram.release()


def _prep_inputs(x, router_w, proj_w4, proj_w16, wg, wu, wd):
    xT = np.ascontiguousarray(x.reshape(BT, D).T)                    # [768, 2048]
    wg_r = wg.reshape(E, D, F)
    wu_r = wu.reshape(E, D, F)
    wd_r = wd.reshape(E, F, D)
    esel = np.kron(np.eye(E), np.ones((1, 128))).astype(np.float32)  # [12, 1536]
    onesc = np.ones((128, 1), np.float32)
    ident = np.eye(128, dtype=np.float32)
    in_maps = []
    for c in range(NCORE):
        fs = slice(c * FS, (c + 1) * FS)
        ds = slice(c * DSH, (c + 1) * DSH)
        in_maps.append({
            "xT": xT,
            "rw": np.ascontiguousarray(router_w),
            "pw4": np.ascontiguousarray(proj_w4[:, ds]),
            "pw16": np.ascontiguousarray(proj_w16[:, ds]),
            "wg": np.ascontiguousarray(wg_r[:, :, fs]).reshape(E, DK, 128, FS),
            "wu": np.ascontiguousarray(wu_r[:, :, fs]).reshape(E, DK, 128, FS),
            "wd": np.ascontiguousarray(wd_r[:, fs, :]).reshape(E, NFH, 128, D),
            "esel": esel, "onesc": onesc, "ident": ident,
        })
    return in_maps


def kernel(x, router_w, proj_w4, proj_w16, wg, wu, wd, _trace=False):
    if "nc" not in _CACHE:
        _CACHE["nc"] = _build()
    nc = _CACHE["nc"]
    in_maps = _prep_inputs(np.asarray(x, np.float32), np.asarray(router_w, np.float32),
                           np.asarray(proj_w4, np.float32), np.asarray(proj_w16, np.float32),
                           np.asarray(wg, np.float32), np.asarray(wu, np.float32),
                           np.asarray(wd, np.float32))
    res = bass_utils.run_bass_kernel_spmd(nc, in_maps, core_ids=list(range(NCORE)),
                                          trace=_trace)
    outT = np.concatenate([res.results[c]["out"] for c in range(NCORE)], axis=0)
    out = np.ascontiguousarray(outT.T).reshape(B, T, D)
    aux = np.float32(res.results[0]["aux"][0, 0])
    _CACHE["last_results"] = res
    return out, aux


# revision 21
# speedup vs baseline: 1.0273x; 1.0273x over previous
"""Trainium2 Bass kernel for nn_MixtureOfGranularities (MoE with multi-scale routing).

Strategy (8 NeuronCores, SPMD single NEFF):
  - FFN-dim (F=2048) sharded 8 ways: each core holds wg/wu/wd slices [*, :, 256]
    for all 12 experts -> no weight replication (28 MB/core HBM traffic).
  - Router + softmax + top-2 + combine computed on every core (tiny, fp32 for
    bit-stable expert selection).
  - Pooling projections (proj_w4 / proj_w16) sharded by output-dim (96 rows per
    core), AllGather'd to every core.
  - Expert matmuls in float32r (tf32-like, 4x faster than fp32 on the PE).
  - Per-core partial outputs (sum over the local F shard) combined with an
    fp32 ReduceScatter; host concatenates the 8 disjoint shards.
"""
import numpy as np

import concourse.bass as bass
import concourse.tile as tile
from concourse import bacc, mybir
from concourse import bass_utils

F32 = mybir.dt.float32
F32R = mybir.dt.float32r
AFT = mybir.ActivationFunctionType
Alu = mybir.AluOpType
AxX = mybir.AxisListType.X

B, T, D, F, S, EPC, E = 2, 1024, 768, 2048, 3, 4, 12
BT = B * T                  # 2048
NCORE = 8
FS = F // NCORE             # 256  per-core FFN shard
NFH = FS // 128             # 2
DK = D // 128               # 6    d k-tiles
NT = 4                      # token tiles of 512
TT = BT // NT               # 512
DSH = D // NCORE            # 96   pooling dout shard rows per core
G4, G16 = BT // 4, BT // 16  # 512, 128 pooled group counts

_CACHE = {}


def _build():
    nc = bacc.Bacc("TRN2", target_bir_lowering=False, debug=False,
                   enable_asserts=False, num_devices=NCORE)

    xT_d = nc.dram_tensor("xT", [D, BT], F32R, kind="ExternalInput").ap()
    rw_d = nc.dram_tensor("rw", [D, E], F32, kind="ExternalInput").ap()
    pw4_d = nc.dram_tensor("pw4", [4 * D, DSH], F32R, kind="ExternalInput").ap()
    pw16_d = nc.dram_tensor("pw16", [16 * D, DSH], F32R, kind="ExternalInput").ap()
    wg_d = nc.dram_tensor("wg", [E, DK, 128, FS], F32R, kind="ExternalInput").ap()
    wu_d = nc.dram_tensor("wu", [E, DK, 128, FS], F32R, kind="ExternalInput").ap()
    wd_d = nc.dram_tensor("wd", [E, NFH, 128, D], F32R, kind="ExternalInput").ap()
    esel_d = nc.dram_tensor("esel", [E, E * 128], F32R, kind="ExternalInput").ap()
    onesc_d = nc.dram_tensor("onesc", [128, 1], F32, kind="ExternalInput").ap()
    ident_d = nc.dram_tensor("ident", [128, 128], F32, kind="ExternalInput").ap()

    out_d = nc.dram_tensor("out", [DSH, BT], F32, kind="ExternalOutput").ap()
    aux_d = nc.dram_tensor("aux", [1, 1], F32, kind="ExternalOutput").ap()

    with tile.TileContext(nc) as tc:
        _emit(nc, tc, xT_d, rw_d, pw4_d, pw16_d, wg_d, wu_d, wd_d,
              esel_d, onesc_d, ident_d, out_d, aux_d)
    nc.compile()
    return nc


def _emit(nc, tc, xT_d, rw_d, pw4_d, pw16_d, wg_d, wu_d, wd_d,
          esel_d, onesc_d, ident_d, out_d, aux_d):
    persist = tc.alloc_tile_pool(name="persist", bufs=1)
    dram = tc.alloc_tile_pool(name="dram", bufs=1, space="DRAM")

    cT_sb = persist.tile([E, BT], F32R)                  # combine^T [12, 2048]
    acc_sb = persist.tile([128, DK, BT], F32)            # partial out^T  6.29 MB
    esel_sb = persist.tile([E, E * 128], F32R)
    nc.sync.dma_start(esel_sb[:], esel_d)
    r4_sb = persist.tile([128, DK, G4], F32R)
    r16_sb = persist.tile([128, DK, G16], F32R)

    r4part = dram.tile([DSH, G4], F32R)
    r16part = dram.tile([DSH, G16], F32R)
    r4full = dram.tile([D, G4], F32R, addr_space="Shared")
    r16full = dram.tile([D, G16], F32R, addr_space="Shared")

    # ============ PHASE R+P: router / combine / aux / pooling (xT resident) ====
    with tc.tile_pool(name="rp_sb", bufs=3) as rsb, \
         tc.tile_pool(name="rp_ps", bufs=2, space="PSUM") as rps:
        xT_sb = rsb.tile([128, DK, BT], F32R, bufs=1)    # 48 KB/part, phase-local
        for k in range(DK):
            nc.sync.dma_start(xT_sb[:, k], xT_d.rearrange("(k p) t -> p k t", p=128)[:, k])
        xTf32 = rsb.tile([128, DK, BT], F32, bufs=1)     # exact fp32 copy for router
        for k in range(DK):
            nc.sync.dma_start(xTf32[:, k],
                              xT_d.bitcast(F32).rearrange("(k p) t -> p k t", p=128)[:, k])
        rw_sb = rsb.tile([128, DK, E], F32, bufs=1)
        nc.sync.dma_start(rw_sb[:], rw_d.rearrange("(k p) e -> p k e", p=128))
        onesc_sb = rsb.tile([128, 1], F32, bufs=1)
        nc.sync.dma_start(onesc_sb[:], onesc_d)
        ident_sb = rsb.tile([128, 128], F32, bufs=1)
        nc.sync.dma_start(ident_sb[:], ident_d)

        # ---- pooling (dout rows sharded per core; data differs per core) ----
        # batched weight loads: 1 DMA for pw4, 4 chunked DMAs for pw16
        pw4_sb = rsb.tile([128, 4 * DK, DSH], F32R, bufs=1)
        nc.scalar.dma_start(pw4_sb[:], pw4_d.rearrange("(kt p) d -> p kt d", p=128))
        r4_ps = rps.tile([DSH, G4], F32, bufs=1)
        for kt in range(4 * DK):
            j, k = kt // DK, kt % DK
            nc.tensor.matmul(r4_ps[:], pw4_sb[:, kt], xT_sb[:, k, j::4],
                             start=(kt == 0), stop=(kt == 4 * DK - 1))
        r4o = rsb.tile([DSH, G4], F32R, tag="r4o")
        nc.vector.tensor_copy(r4o[:], r4_ps[:])
        nc.sync.dma_start(r4part[:], r4o[:])

        r16_ps = rps.tile([DSH, G16], F32, bufs=1)
        for c in range(8):
            pw16_sb = rsb.tile([128, 2 * DK, DSH], F32R, tag="pw16", bufs=2,
                               name=f"pw16_{c}")
            nc.scalar.dma_start(
                pw16_sb[:], pw16_d.rearrange("(kt p) d -> p kt d", p=128)
                [:, c * 2 * DK:(c + 1) * 2 * DK])
            for kk in range(2 * DK):
                kt = c * 2 * DK + kk
                j, k = kt // DK, kt % DK
                nc.tensor.matmul(r16_ps[:], pw16_sb[:, kk], xT_sb[:, k, j::16],
                                 start=(kt == 0), stop=(kt == 16 * DK - 1))
        r16o = rsb.tile([DSH, G16], F32R, tag="r16o")
        nc.vector.tensor_copy(r16o[:], r16_ps[:])
        nc.sync.dma_start(r16part[:], r16o[:])

        # --- logits for all 2048 tokens into one psum bank [128, 16*12] ---
        lg_ps = rps.tile([128, 16 * E], F32, bufs=1)
        for i in range(16):                              # token tiles of 128
            for k in range(DK):
                nc.tensor.matmul(lg_ps[:, i * E:(i + 1) * E],
                                 xTf32[:, k, i * 128:(i + 1) * 128],
                                 rw_sb[:, k], start=(k == 0), stop=(k == DK - 1))
        lg = rsb.tile([128, 16, E], F32, bufs=1)
        nc.vector.tensor_copy(lg[:], lg_ps[:].rearrange("p (i e) -> p i e", e=E))

        def bc(t2):  # [128,16] -> [128,16,12] free-axis broadcast
            return t2[:].unsqueeze(-1).broadcast_to([128, 16, E])

        # --- batched top-2 on logits ---
        m1 = rsb.tile([128, 16], F32, bufs=1)
        nc.vector.tensor_reduce(m1[:].unsqueeze(-1), lg[:], axis=AxX, op=Alu.max)
        eqm = rsb.tile([128, 16, E], F32, bufs=1)
        nc.vector.tensor_tensor(eqm[:], lg[:], bc(m1), op=Alu.is_equal)
        nc.vector.tensor_scalar_mul(eqm[:], eqm[:], -1e9)
        nc.vector.tensor_add(eqm[:], lg[:], eqm[:])      # masked logits
        m2 = rsb.tile([128, 16], F32, bufs=1)
        nc.vector.tensor_reduce(m2[:].unsqueeze(-1), eqm[:], axis=AxX, op=Alu.max)
        sel = rsb.tile([128, 16, E], F32, bufs=1)
        nc.vector.tensor_tensor(sel[:], lg[:], bc(m2), op=Alu.is_ge)
        # --- batched softmax-free combine: E*sel / sum(E*sel) ---
        ex = rsb.tile([128, 16, E], F32, bufs=1)
        nc.scalar.activation(ex[:], lg[:], AFT.Exp)
        esel = rsb.tile([128, 16, E], F32, bufs=1)
        nc.vector.tensor_mul(esel[:], ex[:], sel[:])
        den = rsb.tile([128, 16], F32, bufs=1)
        nc.vector.tensor_reduce(den[:].unsqueeze(-1), esel[:], axis=AxX, op=Alu.add)
        rden = rsb.tile([128, 16], F32, bufs=1)
        nc.vector.reciprocal(rden[:], den[:])
        cmb = rsb.tile([128, 16, E], F32, bufs=1)
        nc.vector.tensor_tensor(cmb[:], esel[:], bc(rden), op=Alu.mult)
        # gate p for aux load stats
        sume = rsb.tile([128, 16], F32, bufs=1)
        nc.vector.tensor_reduce(sume[:].unsqueeze(-1), ex[:], axis=AxX, op=Alu.add)
        rsum = rsb.tile([128, 16], F32, bufs=1)
        nc.vector.reciprocal(rsum[:], sume[:])
        p_all = rsb.tile([128, 16, E], F32, bufs=1)
        nc.vector.tensor_tensor(p_all[:], ex[:], bc(rsum), op=Alu.mult)

        # --- aux = E/(4096*2048) * sum_e (count_e * gatesum_e) ---
        aux_ps = rps.tile([1, 2, 16 * E], F32, bufs=1)
        nc.tensor.matmul(aux_ps[0:1, 0], onesc_sb[:],
                         sel[:].rearrange("p i e -> p (i e)"), start=True, stop=True)
        nc.tensor.matmul(aux_ps[0:1, 1], onesc_sb[:],
                         p_all[:].rearrange("p i e -> p (i e)"), start=True, stop=True)
        auxv = rsb.tile([1, 2, E], F32, bufs=1)
        nc.vector.tensor_reduce(auxv[:].unsqueeze(-1),
                                aux_ps[:].rearrange("o t (i e) -> o t e i", e=E),
                                axis=AxX, op=Alu.add)
        auxm = rsb.tile([1, E], F32, bufs=1)
        nc.vector.tensor_mul(auxm[:], auxv[:, 0], auxv[:, 1])
        auxs = rsb.tile([1, 1], F32, bufs=1)
        nc.vector.tensor_reduce(auxs[:], auxm[:], axis=AxX, op=Alu.add)
        nc.vector.tensor_scalar_mul(auxs[:], auxs[:], float(E) / (BT * 2 * BT))
        nc.sync.dma_start(aux_d, auxs[:])

        # --- transpose combine -> cT [12, 2048] ---
        for i in range(16):
            ct_ps = rps.tile([E, 128], F32, tag="ct")
            nc.tensor.transpose(ct_ps[:], cmb[:, i], ident_sb[:])
            nc.vector.tensor_copy(cT_sb[:, i * 128:(i + 1) * 128], ct_ps[:])


    nc.gpsimd.collective_compute(
        "AllGather", Alu.bypass, replica_groups=[list(range(NCORE))],
        ins=[r4part.opt()], outs=[r4full.opt()])
    nc.gpsimd.collective_compute(
        "AllGather", Alu.bypass, replica_groups=[list(range(NCORE))],
        ins=[r16part.opt()], outs=[r16full.opt()])
    nc.sync.dma_start(r4_sb[:], r4full[:].rearrange("(k p) g -> p k g", p=128))
    nc.sync.dma_start(r16_sb[:], r16full[:].rearrange("(k p) g -> p k g", p=128))

    # ================= PHASE E: experts =================
    with tc.tile_pool(name="w_sb", bufs=1) as wsb, \
         tc.tile_pool(name="e_sb", bufs=2) as esb, \
         tc.tile_pool(name="e_ps", bufs=2, space="PSUM") as eps:
        for s in range(S):
            wg_t, wu_t, wd_t = [], [], []
            for e in range(EPC):
                eg = s * EPC + e
                wg_sb = wsb.tile([128, DK, FS], F32R, tag="wg", bufs=4, name=f"wg{eg}")
                nc.gpsimd.dma_start(wg_sb[:], wg_d[eg].rearrange("k p f -> p k f"))
                wu_sb = wsb.tile([128, DK, FS], F32R, tag="wu", bufs=4, name=f"wu{eg}")
                nc.scalar.dma_start(wu_sb[:], wu_d[eg].rearrange("k p f -> p k f"))
                wd_sb = wsb.tile([128, NFH, D], F32R, tag="wd", bufs=4, name=f"wd{eg}")
                nc.gpsimd.dma_start(wd_sb[:], wd_d[eg].rearrange("k p d -> p k d"))
                wg_t.append(wg_sb); wu_t.append(wu_sb); wd_t.append(wd_sb)

            for t in range(NT):
                # moving operand [128, 512] for this scale/k-tile/token-tile
                if s == 0:
                    xtt = esb.tile([128, DK, TT], F32R, tag="xtt", name=f"xtt{t}")
                    nc.sync.dma_start(
                        xtt[:], xT_d
                        .rearrange("(k p) t -> p k t", p=128)[:, :, t * TT:(t + 1) * TT])
                    rhs = lambda k: xtt[:, k]
                elif s == 1:
                    rhs = lambda k: (r4_sb[:, k, t * 128:(t + 1) * 128]
                                     .unsqueeze(-1).broadcast_to([128, 128, 4]))
                else:
                    rhs = lambda k: (r16_sb[:, k, t * 32:(t + 1) * 32]
                                     .unsqueeze(-1).broadcast_to([128, 32, 16]))

                h_t = []
                for e in range(EPC):
                    eg = s * EPC + e
                    cb_ps = eps.tile([128, TT], F32, tag="cb")
                    nc.tensor.matmul(cb_ps[:], esel_sb[:, eg * 128:(eg + 1) * 128],
                                     cT_sb[:, t * TT:(t + 1) * TT], start=True, stop=True)
                    h_sb = wsb.tile([128, NFH, TT], F32R, tag="h", bufs=5, name=f"h{eg}")
                    h_t.append(h_sb)
                    for fh in range(NFH):
                        g_ps = eps.tile([128, TT], F32, tag="gu", bufs=4, name="g_ps")
                        for k in range(DK):
                            nc.tensor.matmul(g_ps[:], wg_t[e][:, k, fh * 128:(fh + 1) * 128],
                                             rhs(k), start=(k == 0), stop=(k == DK - 1))
                        sg = esb.tile([128, TT], F32, tag="sg")
                        nc.scalar.activation(sg[:], g_ps[:], AFT.Silu)
                        u_ps = eps.tile([128, TT], F32, tag="gu", bufs=4, name="u_ps")
                        for k in range(DK):
                            nc.tensor.matmul(u_ps[:], wu_t[e][:, k, fh * 128:(fh + 1) * 128],
                                             rhs(k), start=(k == 0), stop=(k == DK - 1))
                        t1 = esb.tile([128, TT], F32, tag="t1")
                        nc.vector.tensor_mul(t1[:], sg[:], u_ps[:])
                        nc.vector.tensor_mul(h_sb[:, fh], t1[:], cb_ps[:])
                # down projection accumulated over the 4 experts of this scale
                for dt in range(DK):
                    y_ps = eps.tile([128, TT], F32, tag="y")
                    n_mm = EPC * NFH
                    i_mm = 0
                    for e in range(EPC):
                        for k2 in range(NFH):
                            nc.tensor.matmul(y_ps[:], wd_t[e][:, k2, dt * 128:(dt + 1) * 128],
                                             h_t[e][:, k2],
                                             start=(i_mm == 0), stop=(i_mm == n_mm - 1))
                            i_mm += 1
                    dst = acc_sb[:, dt, t * TT:(t + 1) * TT]
                    if s == 0:
                        nc.vector.tensor_copy(dst, y_ps[:])
                    else:
                        nc.vector.tensor_add(dst, dst, y_ps[:])

    # ================= PHASE OUT: chunked ReduceScatter + store =================
    for t in range(NT):
        accT_c = dram.tile([D, TT], F32, tag="accT", name=f"accT{t}", bufs=NT)
        nc.sync.dma_start(accT_c[:].rearrange("(k p) t -> p k t", p=128),
                          acc_sb[:, :, t * TT:(t + 1) * TT])
        rs_c = dram.tile([DSH, TT], F32, tag="rs_c", name=f"rs{t}", bufs=NT)
        nc.gpsimd.collective_compute(
            "ReduceScatter", Alu.add, replica_groups=[list(range(NCORE))],
            ins=[accT_c.opt()], outs=[rs_c.opt()])
        nc.sync.dma_start(out_d[:, t * TT:(t + 1) * TT], rs_c[:])

    persist.release()
    dram.release()


def _prep_inputs(x, router_w, proj_w4, proj_w16, wg, wu, wd):
    xT = np.ascontiguousarray(x.reshape(BT, D).T)                    # [768, 2048]
    wg_r = wg.reshape(E, D, F)
    wu_r = wu.reshape(E, D, F)
    wd_r = wd.reshape(E, F, D)
    esel = np.kron(np.eye(E), np.ones((1, 128))).astype(np.float32)  # [12, 1536]
    onesc = np.ones((128, 1), np.float32)
    ident = np.eye(128, dtype=np.float32)
    in_maps = []
    for c in range(NCORE):
        fs = slice(c * FS, (c + 1) * FS)
        ds = slice(c * DSH, (c + 1) * DSH)
        in_maps.append({
            "xT": xT,
            "rw": np.ascontiguousarray(router_w),
            "pw4": np.ascontiguousarray(proj_w4[:, ds]),
            "pw16": np.ascontiguousarray(proj_w16[:, ds]),
            "wg": np.ascontiguousarray(wg_r[:, :, fs]).reshape(E, DK, 128, FS),
            "wu": np.ascontiguousarray(wu_r[:, :, fs]).reshape(E, DK, 128, FS),
            "wd": np.ascontiguousarray(wd_r[:, fs, :]).reshape(E, NFH, 128, D),
            "esel": esel, "onesc": onesc, "ident": ident,
        })
    return in_maps


def kernel(x, router_w, proj_w4, proj_w16, wg, wu, wd, _trace=False):
    if "nc" not in _CACHE:
        _CACHE["nc"] = _build()
    nc = _CACHE["nc"]
    in_maps = _prep_inputs(np.asarray(x, np.float32), np.asarray(router_w, np.float32),
                           np.asarray(proj_w4, np.float32), np.asarray(proj_w16, np.float32),
                           np.asarray(wg, np.float32), np.asarray(wu, np.float32),
                           np.asarray(wd, np.float32))
    res = bass_utils.run_bass_kernel_spmd(nc, in_maps, core_ids=list(range(NCORE)),
                                          trace=_trace)
    outT = np.concatenate([res.results[c]["out"] for c in range(NCORE)], axis=0)
    out = np.ascontiguousarray(outT.T).reshape(B, T, D)
    aux = np.float32(res.results[0]["aux"][0, 0])
    _CACHE["last_results"] = res
    return out, aux


# revision 23
# speedup vs baseline: 1.0753x; 1.0467x over previous
"""Trainium2 Bass kernel for nn_MixtureOfGranularities (MoE with multi-scale routing).

Strategy (8 NeuronCores, SPMD single NEFF):
  - FFN-dim (F=2048) sharded 8 ways: each core holds wg/wu/wd slices [*, :, 256]
    for all 12 experts -> no weight replication (28 MB/core HBM traffic).
  - Router + softmax + top-2 + combine computed on every core (tiny, fp32 for
    bit-stable expert selection).
  - Pooling projections (proj_w4 / proj_w16) sharded by output-dim (96 rows per
    core), AllGather'd to every core.
  - Expert matmuls in float32r (tf32-like, 4x faster than fp32 on the PE).
  - Per-core partial outputs (sum over the local F shard) combined with an
    fp32 ReduceScatter; host concatenates the 8 disjoint shards.
"""
import numpy as np

import concourse.bass as bass
import concourse.tile as tile
from concourse import bacc, mybir
from concourse import bass_utils

F32 = mybir.dt.float32
F32R = mybir.dt.float32r
AFT = mybir.ActivationFunctionType
Alu = mybir.AluOpType
AxX = mybir.AxisListType.X

B, T, D, F, S, EPC, E = 2, 1024, 768, 2048, 3, 4, 12
BT = B * T                  # 2048
NCORE = 8
FS = F // NCORE             # 256  per-core FFN shard
NFH = FS // 128             # 2
DK = D // 128               # 6    d k-tiles
NT = 4                      # token tiles of 512
TT = BT // NT               # 512
DSH = D // NCORE            # 96   pooling dout shard rows per core
G4, G16 = BT // 4, BT // 16  # 512, 128 pooled group counts

_CACHE = {}


def _build():
    nc = bacc.Bacc("TRN2", target_bir_lowering=False, debug=False,
                   enable_asserts=False, num_devices=NCORE)

    xT_d = nc.dram_tensor("xT", [D, BT], F32R, kind="ExternalInput").ap()
    rw_d = nc.dram_tensor("rw", [D, E], F32, kind="ExternalInput").ap()
    pw4_d = nc.dram_tensor("pw4", [4 * D, DSH], F32R, kind="ExternalInput").ap()
    pw16_d = nc.dram_tensor("pw16", [16 * D, DSH], F32R, kind="ExternalInput").ap()
    wg_d = nc.dram_tensor("wg", [E, DK, 128, FS], F32R, kind="ExternalInput").ap()
    wu_d = nc.dram_tensor("wu", [E, DK, 128, FS], F32R, kind="ExternalInput").ap()
    wd_d = nc.dram_tensor("wd", [E, NFH, 128, D], F32R, kind="ExternalInput").ap()
    esel_d = nc.dram_tensor("esel", [E, E * 128], F32R, kind="ExternalInput").ap()
    onesc_d = nc.dram_tensor("onesc", [128, 1], F32, kind="ExternalInput").ap()
    ident_d = nc.dram_tensor("ident", [128, 128], F32, kind="ExternalInput").ap()

    out_d = nc.dram_tensor("out", [DSH, BT], F32, kind="ExternalOutput").ap()
    aux_d = nc.dram_tensor("aux", [1, 1], F32, kind="ExternalOutput").ap()

    with tile.TileContext(nc) as tc:
        _emit(nc, tc, xT_d, rw_d, pw4_d, pw16_d, wg_d, wu_d, wd_d,
              esel_d, onesc_d, ident_d, out_d, aux_d)
    nc.compile()
    return nc


def _emit(nc, tc, xT_d, rw_d, pw4_d, pw16_d, wg_d, wu_d, wd_d,
          esel_d, onesc_d, ident_d, out_d, aux_d):
    persist = tc.alloc_tile_pool(name="persist", bufs=1)
    dram = tc.alloc_tile_pool(name="dram", bufs=1, space="DRAM")

    cT_sb = persist.tile([E, BT], F32R)                  # combine^T [12, 2048]
    acc_sb = persist.tile([128, DK, BT], F32)            # partial out^T  6.29 MB
    esel_sb = persist.tile([E, E * 128], F32R)
    nc.sync.dma_start(esel_sb[:], esel_d)
    r4_sb = persist.tile([128, DK, G4], F32R)
    r16_sb = persist.tile([128, DK, G16], F32R)

    r4part = dram.tile([DSH, G4], F32R)
    r16part = dram.tile([DSH, G16], F32R)
    r4full = dram.tile([D, G4], F32R, addr_space="Shared")
    r16full = dram.tile([D, G16], F32R, addr_space="Shared")

    # ============ PHASE R+P: router / combine / aux / pooling (xT resident) ====
    with tc.tile_pool(name="rp_sb", bufs=3) as rsb, \
         tc.tile_pool(name="rp_ps", bufs=2, space="PSUM") as rps:
        xT_sb = rsb.tile([128, DK, BT], F32R, bufs=1)    # 48 KB/part, phase-local
        for k in range(DK):
            nc.sync.dma_start(xT_sb[:, k], xT_d.rearrange("(k p) t -> p k t", p=128)[:, k])
        xTf32 = rsb.tile([128, DK, BT], F32, bufs=1)     # exact fp32 copy for router
        for k in range(DK):
            nc.sync.dma_start(xTf32[:, k],
                              xT_d.bitcast(F32).rearrange("(k p) t -> p k t", p=128)[:, k])
        rw_sb = rsb.tile([128, DK, E], F32, bufs=1)
        nc.sync.dma_start(rw_sb[:], rw_d.rearrange("(k p) e -> p k e", p=128))
        onesc_sb = rsb.tile([128, 1], F32, bufs=1)
        nc.sync.dma_start(onesc_sb[:], onesc_d)
        ident_sb = rsb.tile([128, 128], F32, bufs=1)
        nc.sync.dma_start(ident_sb[:], ident_d)

        # ---- pooling (dout rows sharded per core; data differs per core) ----
        # batched weight loads: 1 DMA for pw4, 4 chunked DMAs for pw16
        pw4_sb = rsb.tile([128, 4 * DK, DSH], F32R, bufs=1)
        nc.scalar.dma_start(pw4_sb[:], pw4_d.rearrange("(kt p) d -> p kt d", p=128))
        r4_ps = rps.tile([DSH, G4], F32, bufs=1)
        for kt in range(4 * DK):
            j, k = kt // DK, kt % DK
            nc.tensor.matmul(r4_ps[:], pw4_sb[:, kt], xT_sb[:, k, j::4],
                             start=(kt == 0), stop=(kt == 4 * DK - 1))
        r4o = rsb.tile([DSH, G4], F32R, tag="r4o")
        nc.vector.tensor_copy(r4o[:], r4_ps[:])
        nc.sync.dma_start(r4part[:], r4o[:])

        r16_ps = rps.tile([DSH, G16], F32, bufs=1)
        for c in range(8):
            pw16_sb = rsb.tile([128, 2 * DK, DSH], F32R, tag="pw16", bufs=2,
                               name=f"pw16_{c}")
            nc.scalar.dma_start(
                pw16_sb[:], pw16_d.rearrange("(kt p) d -> p kt d", p=128)
                [:, c * 2 * DK:(c + 1) * 2 * DK])
            for kk in range(2 * DK):
                kt = c * 2 * DK + kk
                j, k = kt // DK, kt % DK
                nc.tensor.matmul(r16_ps[:], pw16_sb[:, kk], xT_sb[:, k, j::16],
                                 start=(kt == 0), stop=(kt == 16 * DK - 1))
        r16o = rsb.tile([DSH, G16], F32R, tag="r16o")
        nc.vector.tensor_copy(r16o[:], r16_ps[:])
        nc.sync.dma_start(r16part[:], r16o[:])

        # --- logits for all 2048 tokens into one psum bank [128, 16*12] ---
        lg_ps = rps.tile([128, 16 * E], F32, bufs=1)
        for i in range(16):                              # token tiles of 128
            for k in range(DK):
                nc.tensor.matmul(lg_ps[:, i * E:(i + 1) * E],
                                 xTf32[:, k, i * 128:(i + 1) * 128],
                                 rw_sb[:, k], start=(k == 0), stop=(k == DK - 1))
        lg = rsb.tile([128, 16, E], F32, bufs=1)
        nc.vector.tensor_copy(lg[:], lg_ps[:].rearrange("p (i e) -> p i e", e=E))

        def bc(t2):  # [128,16] -> [128,16,12] free-axis broadcast
            return t2[:].unsqueeze(-1).broadcast_to([128, 16, E])

        # --- batched top-2 on logits ---
        m1 = rsb.tile([128, 16], F32, bufs=1)
        nc.vector.tensor_reduce(m1[:].unsqueeze(-1), lg[:], axis=AxX, op=Alu.max)
        eqm = rsb.tile([128, 16, E], F32, bufs=1)
        nc.vector.tensor_tensor(eqm[:], lg[:], bc(m1), op=Alu.is_equal)
        nc.vector.tensor_scalar_mul(eqm[:], eqm[:], -1e9)
        nc.vector.tensor_add(eqm[:], lg[:], eqm[:])      # masked logits
        m2 = rsb.tile([128, 16], F32, bufs=1)
        nc.vector.tensor_reduce(m2[:].unsqueeze(-1), eqm[:], axis=AxX, op=Alu.max)
        sel = rsb.tile([128, 16, E], F32, bufs=1)
        nc.vector.tensor_tensor(sel[:], lg[:], bc(m2), op=Alu.is_ge)
        # --- batched softmax-free combine: E*sel / sum(E*sel) ---
        ex = rsb.tile([128, 16, E], F32, bufs=1)
        nc.scalar.activation(ex[:], lg[:], AFT.Exp)
        esel = rsb.tile([128, 16, E], F32, bufs=1)
        nc.vector.tensor_mul(esel[:], ex[:], sel[:])
        den = rsb.tile([128, 16], F32, bufs=1)
        nc.vector.tensor_reduce(den[:].unsqueeze(-1), esel[:], axis=AxX, op=Alu.add)
        rden = rsb.tile([128, 16], F32, bufs=1)
        nc.vector.reciprocal(rden[:], den[:])
        cmb = rsb.tile([128, 16, E], F32, bufs=1)
        nc.vector.tensor_tensor(cmb[:], esel[:], bc(rden), op=Alu.mult)
        # gate p for aux load stats
        sume = rsb.tile([128, 16], F32, bufs=1)
        nc.vector.tensor_reduce(sume[:].unsqueeze(-1), ex[:], axis=AxX, op=Alu.add)
        rsum = rsb.tile([128, 16], F32, bufs=1)
        nc.vector.reciprocal(rsum[:], sume[:])
        p_all = rsb.tile([128, 16, E], F32, bufs=1)
        nc.vector.tensor_tensor(p_all[:], ex[:], bc(rsum), op=Alu.mult)

        # --- aux = E/(4096*2048) * sum_e (count_e * gatesum_e) ---
        aux_ps = rps.tile([1, 2, 16 * E], F32, bufs=1)
        nc.tensor.matmul(aux_ps[0:1, 0], onesc_sb[:],
                         sel[:].rearrange("p i e -> p (i e)"), start=True, stop=True)
        nc.tensor.matmul(aux_ps[0:1, 1], onesc_sb[:],
                         p_all[:].rearrange("p i e -> p (i e)"), start=True, stop=True)
        auxv = rsb.tile([1, 2, E], F32, bufs=1)
        nc.vector.tensor_reduce(auxv[:].unsqueeze(-1),
                                aux_ps[:].rearrange("o t (i e) -> o t e i", e=E),
                                axis=AxX, op=Alu.add)
        auxm = rsb.tile([1, E], F32, bufs=1)
        nc.vector.tensor_mul(auxm[:], auxv[:, 0], auxv[:, 1])
        auxs = rsb.tile([1, 1], F32, bufs=1)
        nc.vector.tensor_reduce(auxs[:], auxm[:], axis=AxX, op=Alu.add)
        nc.vector.tensor_scalar_mul(auxs[:], auxs[:], float(E) / (BT * 2 * BT))
        nc.sync.dma_start(aux_d, auxs[:])

        # --- transpose combine -> cT [12, 2048] ---
        for i in range(16):
            ct_ps = rps.tile([E, 128], F32, tag="ct")
            nc.tensor.transpose(ct_ps[:], cmb[:, i], ident_sb[:])
            nc.vector.tensor_copy(cT_sb[:, i * 128:(i + 1) * 128], ct_ps[:])


    nc.gpsimd.collective_compute(
        "AllGather", Alu.bypass, replica_groups=[list(range(NCORE))],
        ins=[r4part.opt()], outs=[r4full.opt()])
    nc.gpsimd.collective_compute(
        "AllGather", Alu.bypass, replica_groups=[list(range(NCORE))],
        ins=[r16part.opt()], outs=[r16full.opt()])
    nc.sync.dma_start(r4_sb[:], r4full[:].rearrange("(k p) g -> p k g", p=128))
    nc.sync.dma_start(r16_sb[:], r16full[:].rearrange("(k p) g -> p k g", p=128))

    # ================= PHASE E: experts =================
    with tc.tile_pool(name="w_sb", bufs=1) as wsb, \
         tc.tile_pool(name="e_sb", bufs=2) as esb, \
         tc.tile_pool(name="e_ps", bufs=2, space="PSUM") as eps:
        for s in range(S):
            wg_t, wu_t, wd_t = [], [], []
            for e in range(EPC):
                eg = s * EPC + e
                wg_sb = wsb.tile([128, DK, FS], F32R, tag="wg", bufs=4, name=f"wg{eg}")
                nc.gpsimd.dma_start(wg_sb[:], wg_d[eg].rearrange("k p f -> p k f"))
                wu_sb = wsb.tile([128, DK, FS], F32R, tag="wu", bufs=4, name=f"wu{eg}")
                nc.scalar.dma_start(wu_sb[:], wu_d[eg].rearrange("k p f -> p k f"))
                wd_sb = wsb.tile([128, NFH, D], F32R, tag="wd", bufs=4, name=f"wd{eg}")
                nc.gpsimd.dma_start(wd_sb[:], wd_d[eg].rearrange("k p d -> p k d"))
                wg_t.append(wg_sb); wu_t.append(wu_sb); wd_t.append(wd_sb)

            for t in range(NT):
                # moving operand [128, 512] for this scale/k-tile/token-tile
                if s == 0:
                    xtt = esb.tile([128, DK, TT], F32R, tag="xtt", name=f"xtt{t}")
                    nc.sync.dma_start(
                        xtt[:], xT_d
                        .rearrange("(k p) t -> p k t", p=128)[:, :, t * TT:(t + 1) * TT])
                    rhs = lambda k: xtt[:, k]
                elif s == 1:
                    rhs = lambda k: (r4_sb[:, k, t * 128:(t + 1) * 128]
                                     .unsqueeze(-1).broadcast_to([128, 128, 4]))
                else:
                    rhs = lambda k: (r16_sb[:, k, t * 32:(t + 1) * 32]
                                     .unsqueeze(-1).broadcast_to([128, 32, 16]))

                h_t = []
                for e in range(EPC):
                    eg = s * EPC + e
                    cb_ps = eps.tile([128, TT], F32, tag="cb")
                    nc.tensor.matmul(cb_ps[:], esel_sb[:, eg * 128:(eg + 1) * 128],
                                     cT_sb[:, t * TT:(t + 1) * TT], start=True, stop=True)
                    h_sb = wsb.tile([128, NFH, TT], F32R, tag="h", bufs=5, name=f"h{eg}")
                    h_t.append(h_sb)
                    for fh in range(NFH):
                        g_ps = eps.tile([128, TT], F32, tag="gu", bufs=4, name="g_ps")
                        for k in range(DK):
                            nc.tensor.matmul(g_ps[:], wg_t[e][:, k, fh * 128:(fh + 1) * 128],
                                             rhs(k), start=(k == 0), stop=(k == DK - 1))
                        sg = esb.tile([128, TT], F32, tag="sg")
                        nc.scalar.activation(sg[:], g_ps[:], AFT.Silu)
                        u_ps = eps.tile([128, TT], F32, tag="gu", bufs=4, name="u_ps")
                        for k in range(DK):
                            nc.tensor.matmul(u_ps[:], wu_t[e][:, k, fh * 128:(fh + 1) * 128],
                                             rhs(k), start=(k == 0), stop=(k == DK - 1))
                        t1 = esb.tile([128, TT], F32, tag="t1")
                        nc.vector.tensor_mul(t1[:], sg[:], u_ps[:])
                        nc.vector.tensor_mul(h_sb[:, fh], t1[:], cb_ps[:])
                # down projection accumulated over the 4 experts of this scale
                for dt in range(DK):
                    y_ps = eps.tile([128, TT], F32, tag="y")
                    n_mm = EPC * NFH
                    i_mm = 0
                    for e in range(EPC):
                        for k2 in range(NFH):
                            nc.tensor.matmul(y_ps[:], wd_t[e][:, k2, dt * 128:(dt + 1) * 128],
                                             h_t[e][:, k2],
                                             start=(i_mm == 0), stop=(i_mm == n_mm - 1))
                            i_mm += 1
                    dst = acc_sb[:, dt, t * TT:(t + 1) * TT]
                    if s == 0:
                        nc.vector.tensor_copy(dst, y_ps[:])
                    else:
                        nc.vector.tensor_add(dst, dst, y_ps[:])

    # ================= PHASE OUT: chunked ReduceScatter + store =================
    for t in range(NT):
        accT_c = dram.tile([D, TT], F32, tag="accT", name=f"accT{t}", bufs=NT)
        nc.sync.dma_start(accT_c[:].rearrange("(k p) t -> p k t", p=128),
                          acc_sb[:, :, t * TT:(t + 1) * TT])
        rs_c = dram.tile([DSH, TT], F32, tag="rs_c", name=f"rs{t}", bufs=NT)
        nc.gpsimd.collective_compute(
            "ReduceScatter", Alu.add, replica_groups=[list(range(NCORE))],
            ins=[accT_c.opt()], outs=[rs_c.opt()])
        nc.sync.dma_start(out_d[:, t * TT:(t + 1) * TT], rs_c[:])

    persist.release()
    dram.release()


def _prep_inputs(x, router_w, proj_w4, proj_w16, wg, wu, wd):
    xT = np.ascontiguousarray(x.reshape(BT, D).T)                    # [768, 2048]
    wg_r = wg.reshape(E, D, F)
    wu_r = wu.reshape(E, D, F)
    wd_r = wd.reshape(E, F, D)
    esel = np.kron(np.eye(E), np.ones((1, 128))).astype(np.float32)  # [12, 1536]
    onesc = np.ones((128, 1), np.float32)
    ident = np.eye(128, dtype=np.float32)
    in_maps = []
    for c in range(NCORE):
        fs = slice(c * FS, (c + 1) * FS)
        ds = slice(c * DSH, (c + 1) * DSH)
        in_maps.append({
            "xT": xT,
            "rw": np.ascontiguousarray(router_w),
            "pw4": np.ascontiguousarray(proj_w4[:, ds]),
            "pw16": np.ascontiguousarray(proj_w16[:, ds]),
            "wg": np.ascontiguousarray(wg_r[:, :, fs]).reshape(E, DK, 128, FS),
            "wu": np.ascontiguousarray(wu_r[:, :, fs]).reshape(E, DK, 128, FS),
            "wd": np.ascontiguousarray(wd_r[:, fs, :]).reshape(E, NFH, 128, D),
            "esel": esel, "onesc": onesc, "ident": ident,
        })
    return in_maps


def kernel(x, router_w, proj_w4, proj_w16, wg, wu, wd, _trace=False):
    if "nc" not in _CACHE:
        _CACHE["nc"] = _build()
    nc = _CACHE["nc"]
    in_maps = _prep_inputs(np.asarray(x, np.float32), np.asarray(router_w, np.float32),
                           np.asarray(proj_w4, np.float32), np.asarray(proj_w16, np.float32),
                           np.asarray(wg, np.float32), np.asarray(wu, np.float32),
                           np.asarray(wd, np.float32))
    res = bass_utils.run_bass_kernel_spmd(nc, in_maps, core_ids=list(range(NCORE)),
                                          trace=_trace)
    outT = np.concatenate([res.results[c]["out"] for c in range(NCORE)], axis=0)
    out = np.ascontiguousarray(outT.T).reshape(B, T, D)
    aux = np.float32(res.results[0]["aux"][0, 0])
    _CACHE["last_results"] = res
    return out, aux
